# revision 18
# baseline (speedup 1.0000x reference)
"""Gemma4 vision pooler (segment mean) as a Trainium2 Bass kernel.

For the real inputs this is a 2x2 average-pool over a 64x64 patch grid:
  hs [16, 4096, 1152] f32  ->  out [16, 1024, 1152] f32, scaled by sqrt(1152),
plus a counts>0 bool mask [16, 1024].

Sharding: pure data parallel, 2 batch elements per core on 8 cores.

Device kernel layout (per core, per batch):
  - view hs[b] as a [64 (y), 64 (x), 1152 (h)] grid of patch vectors
  - SBUF tile partition p = (yk, q): yk in [0,32) output row, q in [0,4) x-quarter
  - free dims = [y in {2yk, 2yk+1}, x 4 tokens, h 1152]
  - stage 1: strided tensor_add folds x-pairs   -> [128, 2, 2, 1152]
  - stage 2: tensor_add folds the y-pair        -> [128, 2, 1152]
  - ScalarE multiplies by sqrt(1152)/4, contiguous DMA store
All pooling happens along the free dimension; DMA chunks are >=18KB contiguous.
"""

import math
import os

import numpy as np

_B, _L, _H = 16, 4096, 1152
_OUT = 1024
_NCORES = 8
_BPC = _B // _NCORES  # batches per core

# Set by the last device run (BassKernelResults); test.py reads these.
LAST_RESULTS = None


def _build_fast_nc():
    import concourse.bacc as bacc
    import concourse.mybir as mybir
    from concourse.tile import TileContext

    # Bacc (not plain Bass): its compile pipeline runs
    # generate_event_semaphores, which splits multi-sem sync waits to meet
    # the TRN2 one-wait-per-instruction hardware constraint.
    nc = bacc.Bacc("TRN2", name="gemma_pool")
    hs = nc.dram_tensor("hs", [_BPC, _L, _H], mybir.dt.float32, kind="ExternalInput")
    out = nc.dram_tensor(
        "out", [_BPC, _OUT, _H], mybir.dt.float32, kind="ExternalOutput"
    )
    scale = math.sqrt(_H) / 4.0

    # Streamed 2x2 pool, 8 iterations per core (2 batches x 4 row-groups):
    #   - iteration (b, jy) covers output rows yk in [8*jy, 8*jy+8);
    #     partition p = yk8*16 + q16 (x-quarters of 4 tokens).
    #   - two 2.36MB HWDGE loads (one per y-row of the pair), 18KB
    #     contiguous runs per partition;
    #   - DVE: x-pair fold of row y0 into o, then two in-place adds of
    #     row y1's x-pairs, then the sqrt(H)/4 scale;
    #   - one 1.18MB HWDGE store per iteration, a fully contiguous slab.
    # Multi-sem waits are legal here: Bacc's generate_event_semaphores
    # splits them to meet the 1-wait-per-instruction HW constraint.
    with TileContext(nc) as tc:
        with (
            tc.tile_pool(name="pin", bufs=4) as pin,
            tc.tile_pool(name="pout", bufs=4) as pout,
        ):
            for b in range(_BPC):
                # token = (16*jy + 2*yk8 + y)*64 + 4*q16 + xx
                src_all = hs[b].rearrange(
                    "(jy yk8 y q16 xx) h -> jy yk8 q16 y xx h",
                    jy=4, yk8=8, y=2, q16=16, xx=4,
                )
                # segment = 256*jy + 32*yk8 + 2*q16 + k
                dst_all = out[b].rearrange(
                    "(jy yk8 q16 k) h -> jy yk8 q16 k h", jy=4, yk8=8, q16=16, k=2
                )
                for jy in range(4):
                    tin0 = pin.tile([128, 4, _H], mybir.dt.float32, tag="tin0")
                    nc.sync.dma_start(out=tin0[:], in_=src_all[jy, :, :, 0])
                    tin1 = pin.tile([128, 4, _H], mybir.dt.float32, tag="tin1")
                    nc.sync.dma_start(out=tin1[:], in_=src_all[jy, :, :, 1])
                    o = pout.tile([128, 2, _H], mybir.dt.float32)
                    nc.vector.tensor_add(
                        out=o[:], in0=tin0[:, 0:4:2, :], in1=tin0[:, 1:4:2, :]
                    )
                    nc.vector.tensor_add(
                        out=o[:], in0=o[:], in1=tin1[:, 0:4:2, :]
                    )
                    nc.vector.tensor_add(
                        out=o[:], in0=o[:], in1=tin1[:, 1:4:2, :]
                    )
                    nc.vector.tensor_scalar_mul(o[:], o[:], scale)
                    nc.sync.dma_start(out=dst_all[jy], in_=o[:])
    # Bacc defers register allocation etc. to its compile pipeline, which
    # finalize() runs; the PJRT exec path requires a finalized module.
    nc.finalize()
    return nc


def _run_spmd(nc, in_maps):
    global LAST_RESULTS
    from concourse import bass_utils

    trace = bool(os.environ.get("GEMMA_POOL_TRACE"))
    res = bass_utils.run_bass_kernel_spmd(
        nc, in_maps, core_ids=list(range(_NCORES)), trace=trace
    )
    LAST_RESULTS = res
    return res.results


def _run_fast(hs16):
    nc = _build_fast_nc()
    in_maps = [
        {"hs": np.ascontiguousarray(hs16[c * _BPC : (c + 1) * _BPC])}
        for c in range(_NCORES)
    ]
    results = _run_spmd(nc, in_maps)
    return np.concatenate([r["out"] for r in results], axis=0)


def _pool_numpy(hs, seg, out_len, k_sq, root_h):
    """General segment-mean fallback, mirrors the jax reference exactly."""
    B, L, H = hs.shape
    out = np.zeros((B, out_len, H), dtype=np.float64)
    for b in range(B):
        valid = (seg[b] >= 0) & (seg[b] < out_len)
        np.add.at(out[b], seg[b][valid], hs[b][valid].astype(np.float64))
    return (out / k_sq * root_h).astype(np.float32)


def kernel(hidden_states, pixel_position_ids, padding_positions, output_length):
    hs = np.asarray(hidden_states, dtype=np.float32)
    pos = np.asarray(pixel_position_ids)
    pad = np.asarray(padding_positions)
    out_len = int(np.asarray(output_length))
    B, L, H = hs.shape
    root_h = float(H) ** 0.5

    if L == out_len:
        # No pooling: just scale; mask passes through unchanged.
        return (hs * np.float32(root_h)), pad

    k = int((L // out_len) ** 0.5)
    k_sq = k * k
    assert k_sq * out_len == L

    clamped = np.maximum(pos, 0)
    max_x = clamped[..., 0].max(axis=-1) + 1  # [B]
    kidx = clamped // k
    seg = (kidx[..., 0] + (max_x // k)[:, None] * kidx[..., 1]).astype(np.int64)

    counts = np.zeros((B, out_len), dtype=np.int64)
    for b in range(B):
        v = (seg[b] >= 0) & (seg[b] < out_len)
        counts[b] = np.bincount(seg[b][v], minlength=out_len)[:out_len]
    mask = counts > 0

    if pad.any():
        hs = np.where(pad[..., None], np.float32(0), hs)

    idx = np.arange(L)
    exp_seg = (idx % 64) // 2 + 32 * ((idx // 64) // 2)
    fast = (
        B == _B
        and L == _L
        and H == _H
        and out_len == _OUT
        and k == 2
        and bool((seg == exp_seg[None, :]).all())
    )
    if fast:
        out = _run_fast(hs)
    else:
        out = _pool_numpy(hs, seg, out_len, k_sq, root_h)
    return out, mask
